# revision 1
# baseline (speedup 1.0000x reference)
"""Trainium2 Bass kernel for nn_FLAttention (B=64, D=512, H=8).

Math (per batch b, head h), with xa = x*sem_w + sem_b:
    qv_q = alpha_q[h]*xa_q + beta_q[h],  kv_k = alpha_k[h]*xa_k
    r_{q,k} = 1/|kv_k - qv_q|            (eps=1e-8 negligible, folded away)
    m_q = max_k r ; e = exp(r - m_q) ; Z_q = sum_k e ; N_q = sum_k e*xa_k
    out_q = xa_q + sum_h (alpha_v[h]/sqrt(H)) * N_q/Z_q + sum_h beta_v[h]/sqrt(H)
(The beta_v term is constant because softmax rows sum to 1.)

Sharding: pure data parallel, 8 batches per core across 8 cores.

Device layout per (b,h): partitions = q (4 tiles of 128), free = k (512).
Per tile: ScalarE Abs (affine folded into per-partition scale/bias),
DVE reciprocal_approx_fast, DVE max-reduce (negated), ScalarE Exp
(bias=-max, accum_out -> Z), DVE tensor_tensor_reduce (e*xa -> N).
All per-head/per-batch constants are precomputed host-side in
partition-major layouts so the device never broadcasts or transposes
anything except one tiny PE transpose of the final [128,32] result.
"""

import math
import numpy as np
from contextlib import ExitStack

B, D, H = 64, 512, 8
NCORES = 8
BPC = B // NCORES      # batches per core = 8
P = 128                # partitions
QT = D // P            # q tiles per batch = 4
SQH = math.sqrt(H)

_PROGRAMS = {}


class _nullcm:
    def __init__(self, it):
        self.it = it
    def __enter__(self):
        return None
    def __exit__(self, *a):
        return False

# fraction of tiles whose reciprocal runs on ScalarE as exp(-ln(u)) to
# balance DVE/ScalarE load: every tile with index % RSPLIT == 0
# (HW-measured optimum: every 4th (j,h) group on ScalarE)
RSPLIT = 4
# "group": whole (j,h) groups round-robin on ScalarE per RSPLIT.
# "intra": in EVERY group, the last SC_TILES q-tiles' reciprocal (+ the dmin
# columns) run on ScalarE — same fraction, finer interleave.
SPLIT_MODE = "group"
SC_TILES = 1
# every NSPLIT-th N row-sum runs on ScalarE (Copy+accum) instead of DVE
NSPLIT = 10**9
# xbs PSUM->SBUF copy engine: "sc" or "dve"
XBS_ENGINE = "dve"
# N row-sum path: "gp" = GPSIMD multiply + DVE reduce; "ttr" = one fused
# custom-DVE tensor_tensor_reduce op (measured faster on HW)
NMODE = "ttr"


def _patch_act_tables():
    """Pin Abs/Exp/Ln/Copy/Identity to natural_log_exp_and_others so the
    table-load pass emits one ACT_TABLE_LOAD instead of alternating sets."""
    import functools
    from concourse import bacc, mybir, hw_specs

    if getattr(bacc, "_act_tables_pinned", False):
        return
    A = mybir.ActivationFunctionType
    pin = {A.Abs, A.Exp, A.Ln, A.Copy, A.Identity, A.MemsetZero}
    orig = hw_specs.get_activation_tables

    @functools.cache
    def patched(arch):
        full = orig(arch)
        out = {}
        for name, funcs in full.items():
            if name == "natural_log_exp_and_others":
                out[name] = set(funcs)
            else:
                out[name] = set(funcs) - pin
        return out

    bacc.get_activation_tables = patched
    bacc._act_tables_pinned = True


def _build_program(reps=1, for_i_iters=None):
    """reps: python-unrolled repetitions of the whole body (grading uses 1).
    for_i_iters: if set, wrap the body in a hardware For_i loop with this
    bound — program size is bound-independent, so two builds that differ
    only in for_i_iters have identical NEFF-load cost (clean differential
    timing)."""
    import concourse.bass as bass
    import concourse.tile as tile
    from concourse import bacc, masks, mybir

    _patch_act_tables()

    fp32 = mybir.dt.float32
    nc = bacc.Bacc("TRN2", target_bir_lowering=False, debug=False)

    xrow_d = nc.dram_tensor("xrow", [1, BPC * D], fp32, kind="ExternalInput").ap()
    qbt_d = nc.dram_tensor("qbt", [P, BPC * H * QT], fp32, kind="ExternalInput").ap()
    skp_d = nc.dram_tensor("skp", [P, H], fp32, kind="ExternalInput").ap()
    avp_d = nc.dram_tensor("avp", [P, H * QT], fp32, kind="ExternalInput").ap()
    xap_d = nc.dram_tensor("xap", [P, BPC * QT], fp32, kind="ExternalInput").ap()
    out_d = nc.dram_tensor("out", [BPC * QT, P], fp32, kind="ExternalOutput").ap()

    A = mybir.ActivationFunctionType
    ALU = mybir.AluOpType

    with tile.TileContext(nc) as tc, ExitStack() as ctx:
        const = ctx.enter_context(tc.tile_pool(name="const", bufs=1))
        psum = ctx.enter_context(
            tc.tile_pool(name="psum", bufs=2, space=bass.MemorySpace.PSUM)
        )
        psum_out = ctx.enter_context(
            tc.tile_pool(name="psum_out", bufs=1, space=bass.MemorySpace.PSUM)
        )
        work = ctx.enter_context(tc.tile_pool(name="work", bufs=5))
        nz = ctx.enter_context(tc.tile_pool(name="nz", bufs=2))

        ones = const.tile([1, P], fp32)
        nc.gpsimd.memset(ones[:], 1.0)
        ident = const.tile([P, P], fp32)
        masks.make_identity(nc, ident[:])

        xrow = const.tile([1, BPC * D], fp32)
        nc.gpsimd.dma_start(xrow[:], xrow_d[:])
        qbt = const.tile([P, BPC * H * QT], fp32)
        nc.gpsimd.dma_start(qbt[:], qbt_d[:])
        skp = const.tile([P, H], fp32)
        nc.gpsimd.dma_start(skp[:], skp_d[:])
        avp = const.tile([P, H * QT], fp32)
        nc.gpsimd.dma_start(avp[:], avp_d[:])
        xap = const.tile([P, BPC * QT], fp32)
        nc.gpsimd.dma_start(xap[:], xap_d[:])

        # Final per-q results, columns (j*QT+qt); transposed once at the end.
        outp = const.tile([P, BPC * QT], fp32)

        tile_idx = 0
        rep_cm = (
            tc.For_i(0, for_i_iters, 1)
            if for_i_iters is not None
            else _nullcm(range(reps))
        )
        with rep_cm:
         for rep in range(reps if for_i_iters is None else 1):
          for j in range(BPC):
              # XB[p, f] = xa[b, f] on every partition p (PE outer product).
              xb = psum.tile([P, D], fp32)
              nc.tensor.matmul(
                  xb[:], ones[:], xrow[0:1, j * D : (j + 1) * D], start=True, stop=True
              )
              # SBUF copy for GPSIMD (which cannot read PSUM).
              xbs = work.tile([P, D], fp32, tag="xbs")
              if XBS_ENGINE == "sc":
                  nc.scalar.copy(xbs[:], xb[:])
              else:
                  nc.vector.tensor_copy(xbs[:], xb[:])
              z32 = nz.tile([P, H * QT], fp32)
              n32 = nz.tile([P, H * QT], fp32)
              for h in range(H):
                  # u2big: 4 clamped |d| tiles side by side + their dmin columns
                  u2big = work.tile([P, QT * D + QT], fp32, tag="u2big")
                  for qt in range(QT):
                      col = (j * H + h) * QT + qt
                      u = work.tile([P, D], fp32)
                      nc.scalar.activation(
                          u[:],
                          xb[:],
                          A.Abs,
                          bias=qbt[:, col : col + 1],
                          scale=skp[:, h : h + 1],
                      )
                      # u2 = max(u, eps) elementwise; dmin = min_k u2 (row min)
                      nc.vector.tensor_scalar(
                          out=u2big[:, qt * D : (qt + 1) * D],
                          in0=u[:],
                          scalar1=1e-8,
                          scalar2=3.0e38,
                          op0=ALU.max,
                          op1=ALU.min,
                          accum_out=u2big[:, QT * D + qt : QT * D + qt + 1],
                      )
                      tile_idx += 1
                  # one reciprocal for all 4 tiles AND the 4 dmin scalars;
                  # row max of r == recip(dmin) elementwise-exactly
                  rbig = work.tile([P, QT * D + QT], fp32, tag="rbig")
                  mneg4 = work.tile([P, QT], fp32, tag="mneg4")
                  if SPLIT_MODE == "intra":
                      cut = (QT - SC_TILES) * D
                      nc.vector.reciprocal_approx_fast(
                          rbig[:, 0:cut], u2big[:, 0:cut]
                      )
                      # ScalarE r = exp(-ln(u)) for the tail tiles + dmin cols
                      nc.scalar.activation(
                          rbig[:, cut:], u2big[:, cut:], A.Ln
                      )
                      nc.scalar.activation(
                          rbig[:, cut:], rbig[:, cut:], A.Exp, scale=-1.0
                      )
                      # m must dominate BOTH recip implementations' images of
                      # dmin (they disagree by ~4e-6 rel; x r~1e8 that would
                      # overflow exp) -> take the max of the two, then negate
                      mdve = work.tile([P, QT], fp32, tag="mdve")
                      nc.vector.reciprocal_approx_fast(
                          mdve[:], u2big[:, QT * D :]
                      )
                      nc.vector.tensor_max(
                          mneg4[:], mdve[:], rbig[:, QT * D : QT * D + QT]
                      )
                      nc.vector.tensor_scalar_mul(mneg4[:], mneg4[:], -1.0)
                  else:
                      if (j * H + h) % RSPLIT == 0:
                          # ScalarE route r = exp(-ln(u)) offloads DVE; second
                          # activation is in-place (write trails read)
                          nc.scalar.activation(rbig[:], u2big[:], A.Ln)
                          nc.scalar.activation(
                              rbig[:], rbig[:], A.Exp, scale=-1.0
                          )
                      else:
                          nc.vector.reciprocal_approx_fast(rbig[:], u2big[:])
                      nc.vector.tensor_scalar_mul(
                          mneg4[:], rbig[:, QT * D : QT * D + QT], -1.0
                      )
                  for qt in range(QT):
                      zc = qt * H + h
                      e = work.tile([P, D], fp32, tag=f"e{qt}")
                      nc.scalar.activation(
                          e[:],
                          rbig[:, qt * D : (qt + 1) * D],
                          A.Exp,
                          bias=mneg4[:, qt : qt + 1],
                          scale=1.0,
                          accum_out=z32[:, zc : zc + 1],
                      )
                      en = work.tile([P, D], fp32, tag=f"en{qt}")
                      if NMODE == "ttr":
                          from concourse.dve_ops import TENSOR_TENSOR_REDUCE
                          nc.vector._custom_dve(
                              TENSOR_TENSOR_REDUCE,
                              out=en[:],
                              in0=e[:],
                              in1=xbs[:],
                              s0=0.0,
                              s1=1.0,
                              imm2=0.0,
                              accum_out=n32[:, zc : zc + 1],
                          )
                          continue
                      nc.gpsimd.tensor_mul(en[:], e[:], xbs[:])
                      if zc % NSPLIT == 0:
                          # ScalarE copy-with-accum as the row-sum (load
                          # balance); in-place copy, only the accum matters
                          nc.scalar.activation(
                              en[:],
                              en[:],
                              A.Copy,
                              accum_out=n32[:, zc : zc + 1],
                          )
                      else:
                          nc.vector.tensor_reduce(
                              n32[:, zc : zc + 1],
                              en[:],
                              axis=mybir.AxisListType.X,
                              op=ALU.add,
                          )
              # combine: out_q = xa_q + cbeta + sum_h avp * N/Z
              rz = nz.tile([P, H * QT], fp32)
              nc.vector.reciprocal(rz[:], z32[:])
              ratio = nz.tile([P, H * QT], fp32)
              nc.vector.tensor_mul(ratio[:], n32[:], rz[:])
              scaled = nz.tile([P, H * QT], fp32)
              nc.vector.tensor_mul(scaled[:], ratio[:], avp[:])
              acc = nz.tile([P, QT], fp32)
              nc.vector.tensor_reduce(
                  acc[:],
                  scaled[:].rearrange("p (qt h) -> p qt h", qt=QT, h=H),
                  axis=mybir.AxisListType.X,
                  op=ALU.add,
              )
              nc.vector.tensor_add(
                  outp[:, j * QT : (j + 1) * QT],
                  acc[:],
                  xap[:, j * QT : (j + 1) * QT],
              )

        outt = psum_out.tile([BPC * QT, P], fp32)
        nc.tensor.transpose(outt[:], outp[:], ident[:])
        outsb = const.tile([BPC * QT, P], fp32)
        nc.vector.tensor_copy(outsb[:], outt[:])
        nc.gpsimd.dma_start(out_d[:], outsb[:])

    nc.compile()
    return nc


def _get_program(reps=1, for_i_iters=None):
    key = (reps, for_i_iters)
    if key not in _PROGRAMS:
        _PROGRAMS[key] = _build_program(reps, for_i_iters)
    return _PROGRAMS[key]


def _make_in_maps(x, alpha_q, alpha_k, alpha_v, beta_q, beta_v, sem_w, sem_b):
    f = np.float32
    x = np.asarray(x, f)
    aq = np.asarray(alpha_q, f).reshape(H)
    ak = np.asarray(alpha_k, f).reshape(H)
    av = np.asarray(alpha_v, f).reshape(H)
    bq = np.asarray(beta_q, f).reshape(H)
    bv = np.asarray(beta_v, f).reshape(H)
    sw = np.asarray(sem_w, f).reshape(D)
    sb = np.asarray(sem_b, f).reshape(D)

    xa = x * sw + sb  # [B, D]
    cbeta = bv.sum() / SQH

    skp = np.tile(ak, (P, 1)).astype(f)  # [P, H]
    avp = np.zeros((P, H * QT), f)
    for qt in range(QT):
        for h in range(H):
            avp[:, qt * H + h] = av[h] / SQH

    in_maps = []
    for c in range(NCORES):
        bs = slice(c * BPC, (c + 1) * BPC)
        xa_c = xa[bs]  # [BPC, D]
        # xa in partition-major per (j, qt): [P, j, qt]
        xa_pm = xa_c.reshape(BPC, QT, P).transpose(2, 0, 1)  # [P, BPC, QT]
        qbt = np.empty((P, BPC, H, QT), f)
        for h in range(H):
            qbt[:, :, h, :] = -(aq[h] * xa_pm + bq[h])
        xap = (xa_pm + cbeta).reshape(P, BPC * QT).astype(f)
        in_maps.append(
            {
                "xrow": np.ascontiguousarray(xa_c.reshape(1, BPC * D)),
                "qbt": np.ascontiguousarray(qbt.reshape(P, BPC * H * QT)),
                "skp": skp,
                "avp": avp,
                "xap": np.ascontiguousarray(xap),
            }
        )
    return in_maps


def _assemble(results):
    f = np.float32
    out = np.empty((B, D), f)
    for c in range(NCORES):
        o = np.asarray(results[c]["out"], f)  # [BPC*QT, P]
        o = o.reshape(BPC, QT, P).reshape(BPC, D)
        out[c * BPC : (c + 1) * BPC] = o
    return out


def kernel(x, alpha_q, alpha_k, alpha_v, beta_q, beta_v, sem_w, sem_b):
    from concourse.bass_utils import run_bass_kernel_spmd

    in_maps = _make_in_maps(
        x, alpha_q, alpha_k, alpha_v, beta_q, beta_v, sem_w, sem_b
    )
    nc = _get_program()
    res = run_bass_kernel_spmd(nc, in_maps, core_ids=list(range(NCORES)))
    return _assemble(res.results)


def kernel_sim(x, alpha_q, alpha_k, alpha_v, beta_q, beta_v, sem_w, sem_b, core=0):
    """CoreSim (no hardware) single-core check: returns that core's 8 batches."""
    from concourse.bass_interp import CoreSim

    in_maps = _make_in_maps(
        x, alpha_q, alpha_k, alpha_v, beta_q, beta_v, sem_w, sem_b
    )
    nc = _get_program()
    sim = CoreSim(nc, trace=False)
    for name, arr in in_maps[core].items():
        sim.tensor(name)[:] = arr
    sim.simulate(check_with_hw=False)
    o = np.asarray(sim.tensor("out"), np.float32)
    return o.reshape(BPC, QT, P).reshape(BPC, D)

